# revision 1
# baseline (speedup 1.0000x reference)
"""DotProductPredictor kernel for trn2 (8 NeuronCores, SPMD).

Computes per-edge dot products score[e] = <h[src[e]], h[dst[e]]> over 600k
edges against a 100k x 128 fp32 node table, then outputs
(score != global_min(score)) as float32 [600000, 1] — exactly what the
reference's min-max normalize + (norm==0 ? 0 : 1) threshold produces.

Device strategy: edges sharded 8-way data-parallel; h replicated. Row
gathers use the GPSIMD dma_gather custom instruction (int16 indices), so h
is split into 4 banks of 25000 rows and each core's edges are grouped by
(src_bank, dst_bank) into 16 groups of a fixed 5120-edge capacity (padded
with duplicate in-group edges; duplicates can't change the min). Each
1024-edge chunk is two bank-local dma_gathers (the ucode caps at 1024
indices/instruction) spread over 4 SWDGE queues, then a DVE multiply +
per-edge reduce; h is cast to bf16 host-side (the min-gap 2.67 dwarfs bf16
noise ~0.05, and the threshold compares our own f32 scores to their own
min, so argmin is preserved). Global min via per-core reduce_min +
cross-core AllReduce(min); threshold on device with
tensor_scalar(not_equal). Measured ~476-498us HW on 8 cores.
"""

import os

import numpy as np

from concourse import bass, mybir, tile
from concourse import library_config
from concourse.bass_utils import run_bass_kernel_spmd

P = 128            # SBUF partitions
D = 128            # feature dim (one h row = 512B)
N_NODES = 100000
E_TOTAL = 600000
N_CORES = 8
EPC = E_TOTAL // N_CORES       # 75000 edges per core

N_BANKS = 4
BANK = N_NODES // N_BANKS      # 25000 rows per bank (< 32768 => int16 ok)
N_GROUPS = N_BANKS * N_BANKS   # 16 (src_bank, dst_bank) groups
GROUP_CAP = 5120               # fixed per-group slot allocation (mean 4687)
CHUNK = int(os.environ.get("KERNEL_CHUNK", "1024"))  # dma_gather caps at 1024 idx
CB = CHUNK // P                # 8 score blocks per chunk
N_CHUNKS = N_GROUPS * GROUP_CAP // CHUNK   # 80
SLOTS = N_GROUPS * GROUP_CAP   # 81920 padded edge slots per core
SCORE_COLS = SLOTS // P        # 640
IDX_COLS = CHUNK // 16         # 64 int16 columns per gather
N_GATHERS = 2 * N_CHUNKS       # 160

_CACHE = {}


N_SWDGE_QUEUES = int(os.environ.get("KERNEL_SWDGE_QUEUES", "4"))
H_BF16 = os.environ.get("KERNEL_H_BF16", "1") == "1"
SINGLE_PACKET = os.environ.get("KERNEL_SINGLE_PACKET", "1") == "1"
DMA_SCRATCH = int(os.environ.get("KERNEL_DMA_SCRATCH", "16384"))
GBUFS = int(os.environ.get("KERNEL_GBUFS", "6"))
MBUFS = int(os.environ.get("KERNEL_MBUFS", "4"))


def build_nc():
    nc = bass.Bass(
        num_devices=N_CORES,
        num_swdge_queues=N_SWDGE_QUEUES,
        dynamic_dma_scratch_size=DMA_SCRATCH,
    )
    h_dt = mybir.dt.bfloat16 if H_BF16 else mybir.dt.float32
    h = nc.dram_tensor("h", [N_NODES, D], h_dt, kind="ExternalInput")
    idx = nc.dram_tensor(
        "idx", [P, N_GATHERS * IDX_COLS], mybir.dt.int16, kind="ExternalInput"
    )
    out = nc.dram_tensor("out", [P, SCORE_COLS], mybir.dt.float32,
                         kind="ExternalOutput")
    sc_out = nc.dram_tensor("sc", [P, SCORE_COLS], mybir.dt.float32,
                            kind="ExternalOutput")
    pmin_d = nc.dram_tensor("pmin_d", [P, 1], mybir.dt.float32)
    gmin_d = nc.dram_tensor("gmin_d", [P, 1], mybir.dt.float32, addr_space="Shared")

    with tile.TileContext(nc) as tc:
        with (
            tc.tile_pool(name="io", bufs=1) as io_pool,
            tc.tile_pool(name="gs", bufs=GBUFS) as gs_pool,
            tc.tile_pool(name="gd", bufs=GBUFS) as gd_pool,
            tc.tile_pool(name="m", bufs=MBUFS) as m_pool,
        ):
            nc.gpsimd.load_library(library_config.mlp)
            nidx_reg = nc.gpsimd.to_reg(CHUNK)  # one shared count register
            idx_sb = io_pool.tile([P, N_GATHERS * IDX_COLS], mybir.dt.int16)
            nc.sync.dma_start(out=idx_sb[:], in_=idx[:])
            scores = io_pool.tile([P, SCORE_COLS], mybir.dt.float32)

            for ci in range(N_CHUNKS):
                grp = ci * CHUNK // GROUP_CAP
                bs, bd = grp // N_BANKS, grp % N_BANKS
                gs = gs_pool.tile([P, CHUNK], h_dt, tag="gs")
                gd = gd_pool.tile([P, CHUNK], h_dt, tag="gd")
                for side, (g_tile, bank) in enumerate([(gs, bs), (gd, bd)]):
                    gi = 2 * ci + side
                    nc.gpsimd.dma_gather(
                        out_ap=g_tile[:].rearrange("p (b e) -> p b e", e=D),
                        in_ap=h[bank * BANK : (bank + 1) * BANK, :],
                        idxs_ap=idx_sb[:, gi * IDX_COLS : (gi + 1) * IDX_COLS],
                        num_idxs=CHUNK,
                        num_idxs_reg=nidx_reg,
                        elem_size=D,
                        single_packet=SINGLE_PACKET,
                        queue_num=gi % N_SWDGE_QUEUES,
                    )
                m = m_pool.tile([P, CHUNK], mybir.dt.float32, tag="m")
                nc.vector.tensor_tensor(
                    out=m[:], in0=gs[:], in1=gd[:], op=mybir.AluOpType.mult
                )
                nc.vector.tensor_reduce(
                    out=scores[:, ci * CB : (ci + 1) * CB],
                    in_=m[:].rearrange("p (b e) -> p b e", e=D),
                    axis=mybir.AxisListType.X,
                    op=mybir.AluOpType.add,
                )

            pmin = io_pool.tile([P, 1], mybir.dt.float32)
            nc.vector.tensor_reduce(
                out=pmin[:], in_=scores[:], axis=mybir.AxisListType.X,
                op=mybir.AluOpType.min,
            )
            nc.sync.dma_start(out=pmin_d[:], in_=pmin[:])
            if os.environ.get("KERNEL_SKIP_COLLECTIVE", "0") == "1":
                nc.sync.dma_start(out=gmin_d[:], in_=pmin[:])
            else:
                nc.gpsimd.collective_compute(
                    "AllReduce",
                    mybir.AluOpType.min,
                    replica_groups=[list(range(N_CORES))],
                    ins=[pmin_d[:]],
                    outs=[gmin_d[:]],
                )
            # every partition reads all 128 cross-core mins, reduces to the
            # global min so tensor_scalar gets a per-partition scalar operand
            gbc = io_pool.tile([P, P], mybir.dt.float32)
            nc.sync.dma_start(
                out=gbc[:], in_=gmin_d[:, 0][None, :].to_broadcast((P, P))
            )
            gmin = io_pool.tile([P, 1], mybir.dt.float32)
            nc.vector.tensor_reduce(
                out=gmin[:], in_=gbc[:], axis=mybir.AxisListType.X,
                op=mybir.AluOpType.min,
            )
            out_sb = io_pool.tile([P, SCORE_COLS], mybir.dt.float32)
            nc.vector.tensor_scalar(
                out=out_sb[:],
                in0=scores[:],
                scalar1=gmin[:],
                scalar2=None,
                op0=mybir.AluOpType.not_equal,
            )
            nc.sync.dma_start(out=out[:], in_=out_sb[:])
            # debug/safety copy of raw scores — off the critical tail path
            nc.sync.dma_start(out=sc_out[:], in_=scores[:])

    _split_multi_waits(nc)
    # populate .instr bytes of InstISA subclasses (the library-reload pseudo);
    # raw Bass skips this Bacc pass and walrus errors "ISA wrong length"
    mybir.codegen_inst_isa_subclasses(nc)
    return nc


def _split_multi_waits(nc):
    """walrus on this compiler rejects >1 sync-wait command per ISA
    instruction (setupSyncWait: "Too many sync wait commands"). Move all but
    one wait off each instruction onto standalone InstEventSemaphore
    instructions placed immediately before it on the same engine — the
    sequencer blocks on those first, which is semantically identical."""
    n = 0
    for b in nc.m.functions[0].blocks:
        new_list = []
        for ins in b.instructions:
            si = ins.sync_info
            if (
                si is not None
                and si.on_wait
                and len(si.on_wait) > 1
                and not isinstance(ins, mybir.InstEventSemaphore)
            ):
                waits = list(si.on_wait)
                for w in waits[:-1]:
                    n += 1
                    ev = mybir.InstEventSemaphore(
                        name=f"wait_split_{n}",
                        opcode="EventSemaphore",
                        engine=ins.engine,
                        ins=[],
                        outs=[],
                        sync_info=mybir.SyncInfo(on_wait=[w], on_update=[]),
                    )
                    nc.inst_map[ev.name] = ev
                    new_list.append(ev)
                si.on_wait = [waits[-1]]
            new_list.append(ins)
        b.instructions[:] = new_list


def _plan_core(src, dst):
    """Group this core's edges by (src_bank, dst_bank) with fixed caps.

    Returns (idx16 [P, N_GATHERS*IDX_COLS], slot_of_edge [n], overflow list
    of (orig_pos, src, dst))."""
    n = src.shape[0]
    gkey = (src // BANK) * N_BANKS + (dst // BANK)
    order = np.argsort(gkey, kind="stable")
    counts = np.bincount(gkey, minlength=N_GROUPS)
    force_host = bool(counts.min() == 0)  # fabricated pad could corrupt min
    # per-group kept edges (in sorted order) and overflow spill
    kept_sorted = []
    overflow = []
    starts = np.zeros(N_GROUPS + 1, np.int64)
    np.cumsum(counts, out=starts[1:])
    src_slots = np.empty(SLOTS, np.int32)  # bank-local src index per slot
    dst_slots = np.empty(SLOTS, np.int32)
    slot_of_edge = np.full(n, -1, np.int64)
    for g in range(N_GROUPS):
        bs, bd = g // N_BANKS, g % N_BANKS
        members = order[starts[g] : starts[g + 1]]
        if len(members) > GROUP_CAP:
            for pos in members[GROUP_CAP:]:
                overflow.append(int(pos))
            members = members[:GROUP_CAP]
        base = g * GROUP_CAP
        k = len(members)
        slot_of_edge[members] = base + np.arange(k)
        sv = src[members] - bs * BANK
        dv = dst[members] - bd * BANK
        if k == 0:
            # fabricated in-bank pad pair; caller must handle via host path
            pad_s, pad_d = 0, 0
        else:
            pad_s, pad_d = sv[0], dv[0]
        src_slots[base : base + k] = sv
        src_slots[base + k : base + GROUP_CAP] = pad_s
        dst_slots[base : base + k] = dv
        dst_slots[base + k : base + GROUP_CAP] = pad_d
    # build idx16: gather gi=2*ci covers src of chunk ci, gi=2*ci+1 dst
    idx16 = np.empty((16, N_GATHERS * IDX_COLS), np.int16)
    for ci in range(N_CHUNKS):
        for side, arr in ((0, src_slots), (1, dst_slots)):
            gi = 2 * ci + side
            vals = arr[ci * CHUNK : (ci + 1) * CHUNK]
            # index i lives at [i % 16, i // 16]
            idx16[:, gi * IDX_COLS : (gi + 1) * IDX_COLS] = (
                vals.reshape(IDX_COLS, 16).T
            )
    idx16_full = np.tile(idx16, (8, 1))  # replicate across the 8 Q7 cores
    return idx16_full, slot_of_edge, overflow, force_host


def refresh_layout():
    """(Re)build padded-slot -> (row, col) maps for the [P, SCORE_COLS]
    outputs. Called at import; call again if module constants are overridden
    (scaled-down tests)."""
    global _ROW_OF_SLOT, _COL_OF_SLOT
    s = np.arange(SLOTS)
    _ROW_OF_SLOT = (s % CHUNK % P).astype(np.int64)
    _COL_OF_SLOT = ((s // CHUNK) * CB + (s % CHUNK) // P).astype(np.int64)


refresh_layout()


def make_in_maps(h, src, dst):
    if H_BF16:
        import ml_dtypes
        h32 = np.ascontiguousarray(
            np.asarray(h, dtype=np.float32).astype(ml_dtypes.bfloat16)
        )
    else:
        h32 = np.ascontiguousarray(np.asarray(h, dtype=np.float32))
    src32 = np.asarray(src, dtype=np.int64)
    dst32 = np.asarray(dst, dtype=np.int64)
    in_maps, plans = [], []
    for c in range(N_CORES):
        s = src32[c * EPC : (c + 1) * EPC]
        d = dst32[c * EPC : (c + 1) * EPC]
        idx16, slot_of_edge, overflow, force_host = _plan_core(s, d)
        in_maps.append({"h": h32, "idx": np.ascontiguousarray(idx16)})
        plans.append((slot_of_edge, overflow, s, d, force_host))
    return in_maps, plans


def assemble_output(results, plans, h):
    outs = []
    any_overflow = any(p[1] or p[4] for p in plans)
    if any_overflow:
        # recompute global min on host including overflow edges
        h32 = np.asarray(h, dtype=np.float32)
        gmin = np.inf
        core_scores = []
        for (slot_of_edge, overflow, s, d, _), r in zip(plans, results):
            sc = r["sc"][_ROW_OF_SLOT[slot_of_edge], _COL_OF_SLOT[slot_of_edge]]
            for pos in overflow:
                sc[pos] = float(h32[s[pos]] @ h32[d[pos]])
            core_scores.append(sc)
            gmin = min(gmin, float(sc.min()))
        for sc in core_scores:
            outs.append((sc != gmin).astype(np.float32))
    else:
        for (slot_of_edge, _, _, _, _), r in zip(plans, results):
            o = r["out"][_ROW_OF_SLOT[slot_of_edge], _COL_OF_SLOT[slot_of_edge]]
            outs.append(o)
    return np.concatenate(outs).reshape(E_TOTAL, 1).astype(np.float32)


def kernel(h, src, dst):
    if "nc" not in _CACHE:
        _CACHE["nc"] = build_nc()
    nc = _CACHE["nc"]
    in_maps, plans = make_in_maps(h, src, dst)
    res = run_bass_kernel_spmd(nc, in_maps, list(range(N_CORES)))
    return assemble_output(res.results, plans, h)



# revision 2
# speedup vs baseline: 1.2103x; 1.2103x over previous
"""DotProductPredictor kernel v2 for trn2 (8 NeuronCores, SPMD).

score[e] = <h[src[e]], h[dst[e]]> over 600k edges, out = (score != min).

v2 halves the dma_gather work vs the two-sided baseline: only the DST rows
are gathered (GPSIMD descriptor generation at ~2.2ns/idx is the machine
bottleneck); the SRC side is eliminated by sharding edges by src range
(12500 nodes/core), uploading the core's src slab feature-major
([128 feat, 12528 nodes] bf16) and letting the PE compute, per canonical
window w (nodes [80w, 80w+128)), the full cross grid
S[slot, n] = <d_slot, h[80w+n]> with the gathered dst rows as the
stationary operand. The gathers run non-transposed on 4 SWDGE queues
(dma_gather transpose=True races across queues -- concurrent gathers
interleave in the shared XBAR and whole tiles land corrupted; a single
queue is correct but serializes at ~8us/gather). Each 128-slot atom is
instead transposed on the PE (identity matmul, bf16 PSUM out) to give the
D^T stationary. A per-slot one-hot over the 128 window columns (iota ==
window-local src, built on DVE in bf16) then extracts score[slot] via
masked multiply + segmented free-axis reduce; the ACT engine downcasts
score PSUM f32 -> bf16 so the DVE mask ops run at 16-bit rate.
Measured ~417us HW on 8 cores (baseline two-sided gather: ~470us).

Slot space: windows (stride 80, 156 of them) x 4 dst banks of 25000 rows
(int16 gather indices stay bank-local) x 128 slots = 79872 slots/core vs
75000±250 real edges; a host greedy packer assigns each edge to a
(window, bank) atom (window must contain its src; ~120 of 600k edges
overflow and are scored on host). Pad slots duplicate a real in-atom edge
so they cannot perturb the min. Global min via per-core reduce_min +
AllReduce(min); threshold on device (not_equal), overflow handled by a
host threshold fallback fed by the always-shipped raw scores.
"""

import os

import numpy as np

from concourse import bass, mybir, tile
from concourse import library_config
from concourse.bass_utils import run_bass_kernel_spmd

P = 128
D = 128
N_NODES = 100000
E_TOTAL = 600000
N_CORES = 8
NPC = N_NODES // N_CORES          # 12500 src nodes per core
BANK = 25000                      # dst bank rows (int16-safe)
N_BANKS = 4
S_WIN = int(os.environ.get("K2_STRIDE", "80"))
W = int(os.environ.get("K2_WINDOWS", "156"))      # multiple of 6
ATOM = 128                        # slots per (window, bank)
SLOTS_PER_BANK = W * ATOM         # 19968
SLOTS = N_BANKS * SLOTS_PER_BANK  # 79872
GCHUNK = 768                      # transpose dma_gather idx cap (1024 hangs)
TILES_PER_BANK = SLOTS_PER_BANK // GCHUNK  # 26
N_GATHERS = N_BANKS * TILES_PER_BANK       # 104
IDX_COLS = GCHUNK // 16           # 48
SCOL = N_BANKS * W                # 624 score columns
WB = 2                            # windows per PSUM/DVE batch
BCOLS = WB * N_BANKS * ATOM       # 1024
SLAB_COLS = S_WIN * (W - 1) + 128  # 12528
PREFETCH = int(os.environ.get("K2_PREFETCH", "2"))
DBUFS = int(os.environ.get("K2_DBUFS", "3"))
N_SWDGE_QUEUES = 4
PAD_SRCL = 255.0                  # never matches iota 0..127 -> score 0

assert W % 6 == 0 and SLAB_COLS >= NPC

_CACHE = {}


def build_nc():
    nc = bass.Bass(
        num_devices=N_CORES,
        num_swdge_queues=N_SWDGE_QUEUES,
        dynamic_dma_scratch_size=int(os.environ.get("K2_SCRATCH", "16384")),
    )
    bf16 = mybir.dt.bfloat16
    f32 = mybir.dt.float32
    h = nc.dram_tensor("h", [N_NODES, D], bf16, kind="ExternalInput")
    slab = nc.dram_tensor("slab", [P, SLAB_COLS], bf16, kind="ExternalInput")
    idx = nc.dram_tensor("idx", [P, N_GATHERS * IDX_COLS], mybir.dt.int16,
                         kind="ExternalInput")
    srcl = nc.dram_tensor("srcl", [P, SCOL], bf16, kind="ExternalInput")
    iota = nc.dram_tensor("iota", [P, BCOLS], bf16, kind="ExternalInput")
    ident = nc.dram_tensor("ident", [P, P], bf16, kind="ExternalInput")
    out = nc.dram_tensor("out", [P, SCOL], f32, kind="ExternalOutput")
    sc_out = nc.dram_tensor("sc", [P, SCOL], f32, kind="ExternalOutput")
    pmin_d = nc.dram_tensor("pmin_d", [P, 1], f32)
    gmin_d = nc.dram_tensor("gmin_d", [P, 1], f32, addr_space="Shared")

    with tile.TileContext(nc) as tc:
        with (
            tc.tile_pool(name="io", bufs=1) as io_pool,
            tc.tile_pool(name="d0", bufs=DBUFS) as d0_pool,
            tc.tile_pool(name="d1", bufs=DBUFS) as d1_pool,
            tc.tile_pool(name="d2", bufs=DBUFS) as d2_pool,
            tc.tile_pool(name="d3", bufs=DBUFS) as d3_pool,
            tc.tile_pool(name="ps", bufs=2, space="PSUM") as ps_pool,
            tc.tile_pool(name="psT", bufs=2, space="PSUM") as psT_pool,
            tc.tile_pool(name="dT0", bufs=DBUFS) as dT0_pool,
            tc.tile_pool(name="dT1", bufs=DBUFS) as dT1_pool,
            tc.tile_pool(name="dT2", bufs=DBUFS) as dT2_pool,
            tc.tile_pool(name="dT3", bufs=DBUFS) as dT3_pool,
            tc.tile_pool(name="sbf", bufs=2) as sbf_pool,
            tc.tile_pool(name="msk", bufs=2) as msk_pool,
            tc.tile_pool(name="mm", bufs=2) as mm_pool,
        ):
            d_pools = [d0_pool, d1_pool, d2_pool, d3_pool]
            dT_pools = [dT0_pool, dT1_pool, dT2_pool, dT3_pool]
            nc.gpsimd.load_library(library_config.mlp)
            nidx_reg = nc.gpsimd.to_reg(GCHUNK)

            idx_sb = io_pool.tile([P, N_GATHERS * IDX_COLS], mybir.dt.int16)
            nc.sync.dma_start(out=idx_sb[:], in_=idx[:])
            slab_sb = io_pool.tile([P, SLAB_COLS], bf16)
            nc.sync.dma_start(out=slab_sb[:], in_=slab[:])
            srcl_sb = io_pool.tile([P, SCOL], bf16)
            nc.sync.dma_start(out=srcl_sb[:], in_=srcl[:])
            iota_sb = io_pool.tile([P, BCOLS], bf16)
            nc.sync.dma_start(out=iota_sb[:], in_=iota[:])
            ident_sb = io_pool.tile([P, P], bf16)
            nc.sync.dma_start(out=ident_sb[:], in_=ident[:])
            scores = io_pool.tile([P, SCOL], f32)

            d_tiles = {}

            def emit_gathers(t):
                if t >= TILES_PER_BANK:
                    return
                for b in range(N_BANKS):
                    g = d_pools[b].tile([P, GCHUNK], bf16, tag=f"d{b}")
                    gi = N_BANKS * t + b
                    nc.gpsimd.dma_gather(
                        out_ap=g[:].rearrange("p (c e) -> p c e", e=D),
                        in_ap=h[b * BANK:(b + 1) * BANK, :],
                        idxs_ap=idx_sb[:, gi * IDX_COLS:(gi + 1) * IDX_COLS],
                        num_idxs=GCHUNK,
                        num_idxs_reg=nidx_reg,
                        elem_size=D,
                        transpose=False,
                        single_packet=True,
                        queue_num=b,
                    )
                    # transpose each 128-slot atom on PE, downcast to bf16
                    pt = psT_pool.tile([P, GCHUNK], bf16, tag="psT")
                    for a in range(6):
                        nc.tensor.transpose(
                            pt[:, a * ATOM:(a + 1) * ATOM],
                            g[:, a * ATOM:(a + 1) * ATOM],
                            ident_sb[:],
                        )
                    gT = dT_pools[b].tile([P, GCHUNK], bf16, tag=f"dT{b}")
                    if b % 2 == 0:
                        nc.scalar.copy(out=gT[:], in_=pt[:])
                    else:
                        nc.vector.tensor_copy(gT[:], pt[:])
                    d_tiles[(b, t)] = gT

            for t in range(PREFETCH + 1):
                emit_gathers(t)

            ps = None
            for w in range(W):
                t = w // 6
                if w % 6 == 0 and t >= 1:
                    emit_gathers(t + PREFETCH)
                if w % WB == 0:
                    ps = ps_pool.tile([P, BCOLS], f32)
                for b in range(N_BANKS):
                    col = ((w % WB) * N_BANKS + b) * ATOM
                    nc.tensor.matmul(
                        ps[:, col:col + ATOM],
                        d_tiles[(b, t)][:, (w % 6) * ATOM:(w % 6 + 1) * ATOM],
                        slab_sb[:, S_WIN * w:S_WIN * w + ATOM],
                        start=True, stop=True,
                    )
                if w % WB == WB - 1:
                    w0 = w - (WB - 1)
                    nb = WB * N_BANKS
                    s_bf = sbf_pool.tile([P, BCOLS], bf16, tag="sbf")
                    nc.scalar.copy(out=s_bf[:], in_=ps[:])
                    mask = msk_pool.tile([P, BCOLS], bf16, tag="msk")
                    nc.vector.tensor_tensor(
                        out=mask[:].rearrange("p (a n) -> p a n", n=ATOM),
                        in0=iota_sb[:].rearrange("p (a n) -> p a n", n=ATOM),
                        in1=srcl_sb[:, N_BANKS * w0:N_BANKS * w0 + nb]
                            .unsqueeze(2).to_broadcast((P, nb, ATOM)),
                        op=mybir.AluOpType.is_equal,
                    )
                    mm = mm_pool.tile([P, BCOLS], bf16, tag="mm")
                    nc.vector.tensor_tensor(
                        out=mm[:], in0=s_bf[:], in1=mask[:],
                        op=mybir.AluOpType.mult,
                    )
                    nc.vector.tensor_reduce(
                        out=scores[:, N_BANKS * w0:N_BANKS * w0 + nb],
                        in_=mm[:].rearrange("p (a n) -> p a n", n=ATOM),
                        axis=mybir.AxisListType.X,
                        op=mybir.AluOpType.add,
                    )

            pmin = io_pool.tile([P, 1], f32)
            nc.vector.tensor_reduce(
                out=pmin[:], in_=scores[:], axis=mybir.AxisListType.X,
                op=mybir.AluOpType.min,
            )
            nc.sync.dma_start(out=pmin_d[:], in_=pmin[:])
            nc.gpsimd.collective_compute(
                "AllReduce",
                mybir.AluOpType.min,
                replica_groups=[list(range(N_CORES))],
                ins=[pmin_d[:]],
                outs=[gmin_d[:]],
            )
            gbc = io_pool.tile([P, P], f32)
            nc.sync.dma_start(
                out=gbc[:], in_=gmin_d[:, 0][None, :].to_broadcast((P, P))
            )
            gmin = io_pool.tile([P, 1], f32)
            nc.vector.tensor_reduce(
                out=gmin[:], in_=gbc[:], axis=mybir.AxisListType.X,
                op=mybir.AluOpType.min,
            )
            out_sb = io_pool.tile([P, SCOL], f32)
            nc.vector.tensor_scalar(
                out=out_sb[:], in0=scores[:], scalar1=gmin[:], scalar2=None,
                op0=mybir.AluOpType.not_equal,
            )
            nc.sync.dma_start(out=out[:], in_=out_sb[:])
            nc.sync.dma_start(out=sc_out[:], in_=scores[:])

    _split_multi_waits(nc)
    mybir.codegen_inst_isa_subclasses(nc)
    return nc


def _split_multi_waits(nc):
    """walrus rejects >1 sync-wait per ISA instruction; hoist extras onto
    standalone EventSemaphore instructions just before it (same engine)."""
    n = 0
    for blk in nc.m.functions[0].blocks:
        new_list = []
        for ins in blk.instructions:
            si = ins.sync_info
            if (
                si is not None
                and si.on_wait
                and len(si.on_wait) > 1
                and not isinstance(ins, mybir.InstEventSemaphore)
            ):
                waits = list(si.on_wait)
                for wt in waits[:-1]:
                    n += 1
                    ev = mybir.InstEventSemaphore(
                        name=f"wait_split_{n}",
                        opcode="EventSemaphore",
                        engine=ins.engine,
                        ins=[],
                        outs=[],
                        sync_info=mybir.SyncInfo(on_wait=[wt], on_update=[]),
                    )
                    nc.inst_map[ev.name] = ev
                    new_list.append(ev)
                si.on_wait = [waits[-1]]
            new_list.append(ins)
        blk.instructions[:] = new_list


def _plan_core(src_l, dst, epos):
    """Greedy-pack this core's edges into (window, bank) atoms.

    src_l: window-shard-local src (0..NPC-1), dst: global dst, epos: global
    edge positions. Returns (idx16 [128, N_GATHERS*IDX_COLS], srcl
    [128, SCOL] f32, slot_row/col per edge, overflow list, any_empty)."""
    db = dst // BANK
    dl = (dst % BANK).astype(np.int64)
    idx_bank = np.zeros((N_BANKS, SLOTS_PER_BANK), np.int16)
    srcl_arr = np.full((P, SCOL), PAD_SRCL, np.float32)
    slot_row = np.full(src_l.shape[0], -1, np.int64)
    slot_col = np.full(src_l.shape[0], -1, np.int64)
    overflow = []
    any_empty = False
    for b in range(N_BANKS):
        sel = np.nonzero(db == b)[0]
        order = sel[np.argsort(src_l[sel], kind="stable")]
        fill = np.zeros(W, np.int32)
        # per-window member lists
        w_of = np.full(order.shape[0], -1, np.int32)
        for k, e in enumerate(order):
            n = src_l[e]
            w_lo = max(0, -(-(int(n) - 127) // S_WIN))
            w_hi = min(W - 1, int(n) // S_WIN)
            for w in range(w_lo, w_hi + 1):
                if fill[w] < ATOM:
                    w_of[k] = w
                    slot_row[e] = fill[w]
                    slot_col[e] = N_BANKS * w + b
                    srcl_arr[fill[w], N_BANKS * w + b] = n - S_WIN * w
                    idx_bank[b, w * ATOM + fill[w]] = dl[e]
                    fill[w] += 1
                    break
            else:
                overflow.append(int(epos[e]))
        # pad each atom by duplicating its first member (score dup = safe)
        for w in range(W):
            k = fill[w]
            if k == 0:
                any_empty = True  # pad stays (dl=0, srcl=PAD) -> score 0
                continue
            if k < ATOM:
                idx_bank[b, w * ATOM + k:(w + 1) * ATOM] = idx_bank[b, w * ATOM]
                srcl_arr[k:ATOM, N_BANKS * w + b] = srcl_arr[0, N_BANKS * w + b]
    # wrap idx into the ucode's [16, IDX_COLS] layout per gather tile
    idx16 = np.empty((16, N_GATHERS * IDX_COLS), np.int16)
    for t in range(TILES_PER_BANK):
        for b in range(N_BANKS):
            gi = N_BANKS * t + b
            vals = idx_bank[b, t * GCHUNK:(t + 1) * GCHUNK]
            idx16[:, gi * IDX_COLS:(gi + 1) * IDX_COLS] = (
                vals.reshape(IDX_COLS, 16).T
            )
    idx16_full = np.tile(idx16, (8, 1))
    return idx16_full, srcl_arr, slot_row, slot_col, overflow, any_empty


def make_in_maps(h, src, dst):
    import ml_dtypes
    h16 = np.ascontiguousarray(
        np.asarray(h, dtype=np.float32).astype(ml_dtypes.bfloat16)
    )
    src64 = np.asarray(src, dtype=np.int64)
    dst64 = np.asarray(dst, dtype=np.int64)
    iota_arr = np.ascontiguousarray(
        np.broadcast_to((np.arange(BCOLS) % ATOM).astype(ml_dtypes.bfloat16),
                        (P, BCOLS))
    )
    ident_arr = np.ascontiguousarray(np.eye(P, dtype=np.float32)
                                     .astype(ml_dtypes.bfloat16))
    in_maps, plans = [], []
    for c in range(N_CORES):
        epos = np.nonzero((src64 >= c * NPC) & (src64 < (c + 1) * NPC))[0]
        src_l = src64[epos] - c * NPC
        idx16, srcl_arr, slot_row, slot_col, overflow, any_empty = _plan_core(
            src_l, dst64[epos], epos
        )
        slab = np.zeros((P, SLAB_COLS), np.float32)
        slab[:, :NPC] = np.asarray(
            h16[c * NPC:(c + 1) * NPC, :], dtype=np.float32
        ).T
        in_maps.append({
            "h": h16,
            "slab": np.ascontiguousarray(slab.astype(ml_dtypes.bfloat16)),
            "idx": np.ascontiguousarray(idx16),
            "srcl": np.ascontiguousarray(srcl_arr.astype(ml_dtypes.bfloat16)),
            "iota": iota_arr,
            "ident": ident_arr,
        })
        plans.append((epos, slot_row, slot_col, overflow, any_empty))
    return in_maps, plans


def assemble_output(results, plans, h):
    out = np.empty((E_TOTAL,), np.float32)
    all_of = [(c, pos) for c, p in enumerate(plans) for pos in p[3]]
    force_host = bool(all_of) or any(p[4] for p in plans)
    if not force_host:
        for (epos, srow, scol, _, _), r in zip(plans, results):
            out[epos] = r["out"][srow, scol]
        return out.reshape(E_TOTAL, 1)
    # host fallback: recompute global min incl. overflow edges, re-threshold
    import ml_dtypes
    h16 = np.asarray(h, dtype=np.float32).astype(ml_dtypes.bfloat16)
    h32 = np.asarray(h16, dtype=np.float32)
    src, dst = _ASSEMBLE_SRC
    scores = np.empty((E_TOTAL,), np.float32)
    for (epos, srow, scol, overflow, _), r in zip(plans, results):
        scores[epos] = r["sc"][srow, scol]
        for pos in overflow:
            scores[pos] = float(h32[src[pos]] @ h32[dst[pos]])
    gmin = float(scores.min())
    out = (scores != gmin).astype(np.float32)
    return out.reshape(E_TOTAL, 1)


_ASSEMBLE_SRC = [None, None]


def kernel(h, src, dst):
    if "nc" not in _CACHE:
        _CACHE["nc"] = build_nc()
    nc = _CACHE["nc"]
    _ASSEMBLE_SRC[0] = np.asarray(src, dtype=np.int64)
    _ASSEMBLE_SRC[1] = np.asarray(dst, dtype=np.int64)
    in_maps, plans = make_in_maps(h, src, dst)
    res = run_bass_kernel_spmd(nc, in_maps, list(range(N_CORES)))
    return assemble_output(res.results, plans, h)


# revision 10
# speedup vs baseline: 1.2218x; 1.0095x over previous
"""DotProductPredictor kernel v2 for trn2 (8 NeuronCores, SPMD).

score[e] = <h[src[e]], h[dst[e]]> over 600k edges, out = (score != min).

v2 halves the dma_gather work vs the two-sided baseline: only the DST rows
are gathered (GPSIMD descriptor generation at ~2.2ns/idx is the machine
bottleneck); the SRC side is eliminated by sharding edges by src range
(12500 nodes/core), uploading the core's src slab feature-major
([128 feat, 12528 nodes] bf16) and letting the PE compute, per canonical
window w (nodes [80w, 80w+128)), the full cross grid
S[slot, n] = <d_slot, h[80w+n]> with the gathered dst rows as the
stationary operand. The gathers run non-transposed on 4 SWDGE queues
(dma_gather transpose=True races across queues -- concurrent gathers
interleave in the shared XBAR and whole tiles land corrupted; a single
queue is correct but serializes at ~8us/gather). Each 128-slot atom is
instead transposed on the PE (identity matmul, bf16 PSUM out) to give the
D^T stationary. A per-slot one-hot over the 128 window columns (iota ==
window-local src, built on DVE in bf16) then extracts score[slot] via
masked multiply + segmented free-axis reduce; the ACT engine downcasts
score PSUM f32 -> bf16 so the DVE mask ops run at 16-bit rate.
Measured ~417us HW on 8 cores (baseline two-sided gather: ~470us).

Slot space: windows (stride 80, 156 of them) x 4 dst banks of 25000 rows
(int16 gather indices stay bank-local) x 128 slots = 79872 slots/core vs
75000±250 real edges; a host greedy packer assigns each edge to a
(window, bank) atom (window must contain its src; ~120 of 600k edges
overflow and are scored on host). Pad slots duplicate a real in-atom edge
so they cannot perturb the min. Global min via per-core reduce_min +
AllReduce(min); threshold on device (not_equal), overflow handled by a
host threshold fallback fed by the always-shipped raw scores.
"""

import os

import numpy as np

from concourse import bass, mybir, tile
from concourse import library_config
from concourse.bass_utils import run_bass_kernel_spmd

P = 128
D = 128
N_NODES = 100000
E_TOTAL = 600000
N_CORES = 8
NPC = N_NODES // N_CORES          # 12500 src nodes per core
BANK = 25000                      # dst bank rows (int16-safe)
N_BANKS = 4
S_WIN = int(os.environ.get("K2_STRIDE", "82"))
W = int(os.environ.get("K2_WINDOWS", "152"))
ATOM = 128                        # slots per (window, bank)
SLOTS_PER_BANK = W * ATOM         # 19456
SLOTS = N_BANKS * SLOTS_PER_BANK  # 77824
GCHUNK = 1024                     # nt dma_gather idx cap
APT = GCHUNK // ATOM              # atoms per gather tile (8)
TILES_PER_BANK = SLOTS_PER_BANK // GCHUNK  # 19
N_GATHERS = N_BANKS * TILES_PER_BANK       # 76
IDX_COLS = GCHUNK // 16           # 64
SCOL = N_BANKS * W                # 608 score columns
WB = 2                            # windows per PSUM/DVE batch
BCOLS = WB * N_BANKS * ATOM       # 1024
SLAB_COLS = S_WIN * (W - 1) + 128  # 12528
PREFETCH = int(os.environ.get("K2_PREFETCH", "2"))
DBUFS = int(os.environ.get("K2_DBUFS", "3"))
N_SWDGE_QUEUES = 4
PAD_SRCL = 255.0                  # never matches iota 0..127 -> score 0

assert W % APT == 0 and W % WB == 0 and SLAB_COLS >= NPC

_CACHE = {}


def build_nc():
    nc = bass.Bass(
        num_devices=N_CORES,
        num_swdge_queues=N_SWDGE_QUEUES,
        dynamic_dma_scratch_size=int(os.environ.get("K2_SCRATCH", "16384")),
    )
    bf16 = mybir.dt.bfloat16
    f32 = mybir.dt.float32
    h = nc.dram_tensor("h", [N_NODES, D], bf16, kind="ExternalInput")
    slab = nc.dram_tensor("slab", [P, SLAB_COLS], bf16, kind="ExternalInput")
    idx = nc.dram_tensor("idx", [P, N_GATHERS * IDX_COLS], mybir.dt.int16,
                         kind="ExternalInput")
    mask_d = nc.dram_tensor("mask", [P, SCOL * ATOM], bf16,
                            kind="ExternalInput")
    ident = nc.dram_tensor("ident", [P, P], bf16, kind="ExternalInput")
    out = nc.dram_tensor("out", [P, SCOL], f32, kind="ExternalOutput")
    sc_out = nc.dram_tensor("sc", [P, SCOL], f32, kind="ExternalOutput")
    pmin_d = nc.dram_tensor("pmin_d", [P, 1], f32)
    gmin_d = nc.dram_tensor("gmin_d", [P, 1], f32, addr_space="Shared")

    with tile.TileContext(nc) as tc:
        with (
            tc.tile_pool(name="io", bufs=1) as io_pool,
            tc.tile_pool(name="d0", bufs=DBUFS) as d0_pool,
            tc.tile_pool(name="d1", bufs=DBUFS) as d1_pool,
            tc.tile_pool(name="d2", bufs=DBUFS) as d2_pool,
            tc.tile_pool(name="d3", bufs=DBUFS) as d3_pool,
            tc.tile_pool(name="ps", bufs=2, space="PSUM") as ps_pool,
            tc.tile_pool(name="psT", bufs=2, space="PSUM") as psT_pool,
            tc.tile_pool(name="dT0", bufs=DBUFS) as dT0_pool,
            tc.tile_pool(name="dT1", bufs=DBUFS) as dT1_pool,
            tc.tile_pool(name="dT2", bufs=DBUFS) as dT2_pool,
            tc.tile_pool(name="dT3", bufs=DBUFS) as dT3_pool,
            tc.tile_pool(name="sbf", bufs=2) as sbf_pool,
            tc.tile_pool(name="msk", bufs=2) as msk_pool,
            tc.tile_pool(name="mm", bufs=2) as mm_pool,
        ):
            d_pools = [d0_pool, d1_pool, d2_pool, d3_pool]
            dT_pools = [dT0_pool, dT1_pool, dT2_pool, dT3_pool]
            nc.gpsimd.load_library(library_config.mlp)
            nidx_reg = nc.gpsimd.to_reg(GCHUNK)

            idx_sb = io_pool.tile([P, N_GATHERS * IDX_COLS], mybir.dt.int16)
            nc.sync.dma_start(out=idx_sb[:], in_=idx[:])
            slab_sb = io_pool.tile([P, SLAB_COLS], bf16)
            nc.sync.dma_start(out=slab_sb[:], in_=slab[:])
            ident_sb = io_pool.tile([P, P], bf16)
            nc.sync.dma_start(out=ident_sb[:], in_=ident[:])
            scores = io_pool.tile([P, SCOL], f32)

            d_tiles = {}

            def emit_gathers(t):
                if t >= TILES_PER_BANK:
                    return
                for b in range(N_BANKS):
                    g = d_pools[b].tile([P, GCHUNK], bf16, tag=f"d{b}")
                    gi = N_BANKS * t + b
                    nc.gpsimd.dma_gather(
                        out_ap=g[:].rearrange("p (c e) -> p c e", e=D),
                        in_ap=h[b * BANK:(b + 1) * BANK, :],
                        idxs_ap=idx_sb[:, gi * IDX_COLS:(gi + 1) * IDX_COLS],
                        num_idxs=GCHUNK,
                        num_idxs_reg=nidx_reg,
                        elem_size=D,
                        transpose=False,
                        single_packet=True,
                        queue_num=b,
                    )
                    # transpose each 128-slot atom on PE, downcast to bf16
                    pt = psT_pool.tile([P, GCHUNK], bf16, tag="psT")
                    for a in range(APT):
                        nc.tensor.transpose(
                            pt[:, a * ATOM:(a + 1) * ATOM],
                            g[:, a * ATOM:(a + 1) * ATOM],
                            ident_sb[:],
                        )
                    gT = dT_pools[b].tile([P, GCHUNK], bf16, tag=f"dT{b}")
                    if b % 2 == 0:
                        nc.scalar.copy(out=gT[:], in_=pt[:])
                    else:
                        nc.vector.tensor_copy(gT[:], pt[:])
                    d_tiles[(b, t)] = gT

            for t in range(PREFETCH + 1):
                emit_gathers(t)

            ps = None
            for w in range(W):
                t = w // APT
                if w % APT == 0 and t >= 1:
                    emit_gathers(t + PREFETCH)
                if w % WB == 0:
                    ps = ps_pool.tile([P, BCOLS], f32)
                    mask = msk_pool.tile([P, BCOLS], bf16, tag="msk")
                    nc.sync.dma_start(
                        out=mask[:],
                        in_=mask_d[:, N_BANKS * ATOM * w:
                                   N_BANKS * ATOM * w + BCOLS],
                    )
                for b in range(N_BANKS):
                    col = ((w % WB) * N_BANKS + b) * ATOM
                    nc.tensor.matmul(
                        ps[:, col:col + ATOM],
                        d_tiles[(b, t)][:, (w % APT) * ATOM:
                                        (w % APT + 1) * ATOM],
                        slab_sb[:, S_WIN * w:S_WIN * w + ATOM],
                        start=True, stop=True,
                    )
                if w % WB == WB - 1:
                    w0 = w - (WB - 1)
                    nb = WB * N_BANKS
                    s_bf = sbf_pool.tile([P, BCOLS], bf16, tag="sbf")
                    nc.scalar.copy(out=s_bf[:], in_=ps[:])
                    mm = mm_pool.tile([P, BCOLS], bf16, tag="mm")
                    nc.vector.tensor_tensor(
                        out=mm[:], in0=s_bf[:], in1=mask[:],
                        op=mybir.AluOpType.mult,
                    )
                    nc.vector.tensor_reduce(
                        out=scores[:, N_BANKS * w0:N_BANKS * w0 + nb],
                        in_=mm[:].rearrange("p (a n) -> p a n", n=ATOM),
                        axis=mybir.AxisListType.X,
                        op=mybir.AluOpType.add,
                    )

            pmin = io_pool.tile([P, 1], f32)
            nc.vector.tensor_reduce(
                out=pmin[:], in_=scores[:], axis=mybir.AxisListType.X,
                op=mybir.AluOpType.min,
            )
            nc.sync.dma_start(out=pmin_d[:], in_=pmin[:])
            nc.gpsimd.collective_compute(
                "AllReduce",
                mybir.AluOpType.min,
                replica_groups=[list(range(N_CORES))],
                ins=[pmin_d[:]],
                outs=[gmin_d[:]],
            )
            gbc = io_pool.tile([P, P], f32)
            nc.sync.dma_start(
                out=gbc[:], in_=gmin_d[:, 0][None, :].to_broadcast((P, P))
            )
            gmin = io_pool.tile([P, 1], f32)
            nc.vector.tensor_reduce(
                out=gmin[:], in_=gbc[:], axis=mybir.AxisListType.X,
                op=mybir.AluOpType.min,
            )
            out_sb = io_pool.tile([P, SCOL], f32)
            nc.vector.tensor_scalar(
                out=out_sb[:], in0=scores[:], scalar1=gmin[:], scalar2=None,
                op0=mybir.AluOpType.not_equal,
            )
            nc.sync.dma_start(out=out[:], in_=out_sb[:])
            nc.sync.dma_start(out=sc_out[:], in_=scores[:])

    _split_multi_waits(nc)
    mybir.codegen_inst_isa_subclasses(nc)
    return nc


def _split_multi_waits(nc):
    """walrus rejects >1 sync-wait per ISA instruction; hoist extras onto
    standalone EventSemaphore instructions just before it (same engine)."""
    n = 0
    for blk in nc.m.functions[0].blocks:
        new_list = []
        for ins in blk.instructions:
            si = ins.sync_info
            if (
                si is not None
                and si.on_wait
                and len(si.on_wait) > 1
                and not isinstance(ins, mybir.InstEventSemaphore)
            ):
                waits = list(si.on_wait)
                for wt in waits[:-1]:
                    n += 1
                    ev = mybir.InstEventSemaphore(
                        name=f"wait_split_{n}",
                        opcode="EventSemaphore",
                        engine=ins.engine,
                        ins=[],
                        outs=[],
                        sync_info=mybir.SyncInfo(on_wait=[wt], on_update=[]),
                    )
                    nc.inst_map[ev.name] = ev
                    new_list.append(ev)
                si.on_wait = [waits[-1]]
            new_list.append(ins)
        blk.instructions[:] = new_list


def _plan_core(src_l, dst, epos):
    """Greedy-pack this core's edges into (window, bank) atoms.

    src_l: window-shard-local src (0..NPC-1), dst: global dst, epos: global
    edge positions. Returns (idx16 [128, N_GATHERS*IDX_COLS], srcl
    [128, SCOL] f32, slot_row/col per edge, overflow list, any_empty)."""
    db = dst // BANK
    dl = (dst % BANK).astype(np.int64)
    idx_bank = np.zeros((N_BANKS, SLOTS_PER_BANK), np.int16)
    srcl_arr = np.full((P, SCOL), PAD_SRCL, np.float32)
    slot_row = np.full(src_l.shape[0], -1, np.int64)
    slot_col = np.full(src_l.shape[0], -1, np.int64)
    overflow = []
    any_empty = False
    for b in range(N_BANKS):
        sel = np.nonzero(db == b)[0]
        order = sel[np.argsort(src_l[sel], kind="stable")]
        fill = np.zeros(W, np.int32)
        # per-window member lists
        w_of = np.full(order.shape[0], -1, np.int32)
        for k, e in enumerate(order):
            n = src_l[e]
            w_lo = max(0, -(-(int(n) - 127) // S_WIN))
            w_hi = min(W - 1, int(n) // S_WIN)
            for w in range(w_lo, w_hi + 1):
                if fill[w] < ATOM:
                    w_of[k] = w
                    slot_row[e] = fill[w]
                    slot_col[e] = N_BANKS * w + b
                    srcl_arr[fill[w], N_BANKS * w + b] = n - S_WIN * w
                    idx_bank[b, w * ATOM + fill[w]] = dl[e]
                    fill[w] += 1
                    break
            else:
                overflow.append(int(epos[e]))
        # pad each atom by duplicating its first member (score dup = safe)
        for w in range(W):
            k = fill[w]
            if k == 0:
                any_empty = True  # pad stays (dl=0, srcl=PAD) -> score 0
                continue
            if k < ATOM:
                idx_bank[b, w * ATOM + k:(w + 1) * ATOM] = idx_bank[b, w * ATOM]
                srcl_arr[k:ATOM, N_BANKS * w + b] = srcl_arr[0, N_BANKS * w + b]
    # wrap idx into the ucode's [16, IDX_COLS] layout per gather tile
    idx16 = np.empty((16, N_GATHERS * IDX_COLS), np.int16)
    for t in range(TILES_PER_BANK):
        for b in range(N_BANKS):
            gi = N_BANKS * t + b
            vals = idx_bank[b, t * GCHUNK:(t + 1) * GCHUNK]
            idx16[:, gi * IDX_COLS:(gi + 1) * IDX_COLS] = (
                vals.reshape(IDX_COLS, 16).T
            )
    idx16_full = np.tile(idx16, (8, 1))
    return idx16_full, srcl_arr, slot_row, slot_col, overflow, any_empty


def make_in_maps(h, src, dst):
    import ml_dtypes
    h16 = np.ascontiguousarray(
        np.asarray(h, dtype=np.float32).astype(ml_dtypes.bfloat16)
    )
    src64 = np.asarray(src, dtype=np.int64)
    dst64 = np.asarray(dst, dtype=np.int64)
    ident_arr = np.ascontiguousarray(np.eye(P, dtype=np.float32)
                                     .astype(ml_dtypes.bfloat16))
    in_maps, plans = [], []
    for c in range(N_CORES):
        epos = np.nonzero((src64 >= c * NPC) & (src64 < (c + 1) * NPC))[0]
        src_l = src64[epos] - c * NPC
        idx16, srcl_arr, slot_row, slot_col, overflow, any_empty = _plan_core(
            src_l, dst64[epos], epos
        )
        slab = np.zeros((P, SLAB_COLS), np.float32)
        slab[:, :NPC] = np.asarray(
            h16[c * NPC:(c + 1) * NPC, :], dtype=np.float32
        ).T
        in_maps.append({
            "h": h16,
            "slab": np.ascontiguousarray(slab.astype(ml_dtypes.bfloat16)),
            "idx": np.ascontiguousarray(idx16),
            "mask": np.ascontiguousarray(
                (srcl_arr[:, :, None] == np.arange(ATOM, dtype=np.float32))
                .astype(ml_dtypes.bfloat16).reshape(P, SCOL * ATOM)),
            "ident": ident_arr,
        })
        plans.append((epos, slot_row, slot_col, overflow, any_empty))
    return in_maps, plans


def assemble_output(results, plans, h):
    out = np.empty((E_TOTAL,), np.float32)
    all_of = [(c, pos) for c, p in enumerate(plans) for pos in p[3]]
    force_host = bool(all_of) or any(p[4] for p in plans)
    if not force_host:
        for (epos, srow, scol, _, _), r in zip(plans, results):
            out[epos] = r["out"][srow, scol]
        return out.reshape(E_TOTAL, 1)
    # host fallback: recompute global min incl. overflow edges, re-threshold
    import ml_dtypes
    h16 = np.asarray(h, dtype=np.float32).astype(ml_dtypes.bfloat16)
    h32 = np.asarray(h16, dtype=np.float32)
    src, dst = _ASSEMBLE_SRC
    scores = np.empty((E_TOTAL,), np.float32)
    for (epos, srow, scol, overflow, _), r in zip(plans, results):
        scores[epos] = r["sc"][srow, scol]
        for pos in overflow:
            scores[pos] = float(h32[src[pos]] @ h32[dst[pos]])
    gmin = float(scores.min())
    out = (scores != gmin).astype(np.float32)
    return out.reshape(E_TOTAL, 1)


_ASSEMBLE_SRC = [None, None]


def kernel(h, src, dst):
    if "nc" not in _CACHE:
        _CACHE["nc"] = build_nc()
    nc = _CACHE["nc"]
    _ASSEMBLE_SRC[0] = np.asarray(src, dtype=np.int64)
    _ASSEMBLE_SRC[1] = np.asarray(dst, dtype=np.int64)
    in_maps, plans = make_in_maps(h, src, dst)
    res = run_bass_kernel_spmd(nc, in_maps, list(range(N_CORES)))
    return assemble_output(res.results, plans, h)


# revision 11
# speedup vs baseline: 1.3333x; 1.0913x over previous
"""DotProductPredictor kernel v2 for trn2 (8 NeuronCores, SPMD).

score[e] = <h[src[e]], h[dst[e]]> over 600k edges, out = (score != min).

v2 halves the dma_gather work vs the two-sided baseline: only the DST rows
are gathered (GPSIMD descriptor generation at ~2.2ns/idx is the machine
bottleneck); the SRC side is eliminated by sharding edges by src range
(12500 nodes/core), uploading the core's src slab feature-major
([128 feat, 12528 nodes] bf16) and letting the PE compute, per canonical
window w (nodes [80w, 80w+128)), the full cross grid
S[slot, n] = <d_slot, h[80w+n]> with the gathered dst rows as the
stationary operand. The gathers run non-transposed on 4 SWDGE queues
(dma_gather transpose=True races across queues -- concurrent gathers
interleave in the shared XBAR and whole tiles land corrupted; a single
queue is correct but serializes at ~8us/gather). Each 128-slot atom is
instead transposed on the PE (identity matmul, bf16 PSUM out) to give the
D^T stationary. A per-slot one-hot over the 128 window columns (iota ==
window-local src, built on DVE in bf16) then extracts score[slot] via
masked multiply + segmented free-axis reduce; the ACT engine downcasts
score PSUM f32 -> bf16 so the DVE mask ops run at 16-bit rate.
Measured ~417us HW on 8 cores (baseline two-sided gather: ~470us).

Slot space: windows (stride 80, 156 of them) x 4 dst banks of 25000 rows
(int16 gather indices stay bank-local) x 128 slots = 79872 slots/core vs
75000±250 real edges; a host greedy packer assigns each edge to a
(window, bank) atom (window must contain its src; ~120 of 600k edges
overflow and are scored on host). Pad slots duplicate a real in-atom edge
so they cannot perturb the min. Global min via per-core reduce_min +
AllReduce(min); threshold on device (not_equal), overflow handled by a
host threshold fallback fed by the always-shipped raw scores.
"""

import os

import numpy as np

from concourse import bass, mybir, tile
from concourse import library_config
from concourse.bass_utils import run_bass_kernel_spmd

P = 128
D = 128
N_NODES = 100000
E_TOTAL = 600000
N_CORES = 8
NPC = N_NODES // N_CORES          # 12500 src nodes per core
BANK = 25000                      # dst bank rows (int16-safe)
N_BANKS = 4
S_WIN = int(os.environ.get("K2_STRIDE", "82"))
W = int(os.environ.get("K2_WINDOWS", "152"))
ATOM = 128                        # slots per (window, bank)
SLOTS_PER_BANK = W * ATOM         # 19456
SLOTS = N_BANKS * SLOTS_PER_BANK  # 77824
GCHUNK = 1024                     # nt dma_gather idx cap
APT = GCHUNK // ATOM              # atoms per gather tile (8)
TILES_PER_BANK = SLOTS_PER_BANK // GCHUNK  # 19
N_GATHERS = N_BANKS * TILES_PER_BANK       # 76
IDX_COLS = GCHUNK // 16           # 64
SCOL = N_BANKS * W                # 608 score columns
WB = 2                            # windows per PSUM/DVE batch
BCOLS = WB * N_BANKS * ATOM       # 1024
SLAB_COLS = S_WIN * (W - 1) + 128  # 12528
PREFETCH = int(os.environ.get("K2_PREFETCH", "3"))
DBUFS = int(os.environ.get("K2_DBUFS", "4"))
N_SWDGE_QUEUES = 4
PAD_SRCL = 255.0                  # never matches iota 0..127 -> score 0

assert W % APT == 0 and W % WB == 0 and SLAB_COLS >= NPC

_CACHE = {}


def build_nc():
    nc = bass.Bass(
        num_devices=N_CORES,
        num_swdge_queues=N_SWDGE_QUEUES,
        dynamic_dma_scratch_size=int(os.environ.get("K2_SCRATCH", "16384")),
    )
    bf16 = mybir.dt.bfloat16
    f32 = mybir.dt.float32
    h = nc.dram_tensor("h", [N_NODES, D], bf16, kind="ExternalInput")
    slab = nc.dram_tensor("slab", [P, SLAB_COLS], bf16, kind="ExternalInput")
    idx = nc.dram_tensor("idx", [P, N_GATHERS * IDX_COLS], mybir.dt.int16,
                         kind="ExternalInput")
    mask_d = nc.dram_tensor("mask", [P, SCOL * ATOM], bf16,
                            kind="ExternalInput")
    ident = nc.dram_tensor("ident", [P, P], bf16, kind="ExternalInput")
    out = nc.dram_tensor("out", [P, SCOL], f32, kind="ExternalOutput")
    sc_out = nc.dram_tensor("sc", [P, SCOL], f32, kind="ExternalOutput")
    pmin_d = nc.dram_tensor("pmin_d", [P, 1], f32)
    gmin_d = nc.dram_tensor("gmin_d", [P, 1], f32, addr_space="Shared")

    with tile.TileContext(nc) as tc:
        with (
            tc.tile_pool(name="io", bufs=1) as io_pool,
            tc.tile_pool(name="d0", bufs=DBUFS) as d0_pool,
            tc.tile_pool(name="d1", bufs=DBUFS) as d1_pool,
            tc.tile_pool(name="d2", bufs=DBUFS) as d2_pool,
            tc.tile_pool(name="d3", bufs=DBUFS) as d3_pool,
            tc.tile_pool(name="ps", bufs=3, space="PSUM") as ps_pool,
            tc.tile_pool(name="psT", bufs=2, space="PSUM") as psT_pool,
            tc.tile_pool(name="dT0", bufs=DBUFS) as dT0_pool,
            tc.tile_pool(name="dT1", bufs=DBUFS) as dT1_pool,
            tc.tile_pool(name="dT2", bufs=DBUFS) as dT2_pool,
            tc.tile_pool(name="dT3", bufs=DBUFS) as dT3_pool,
            tc.tile_pool(name="sbf", bufs=3) as sbf_pool,
            tc.tile_pool(name="msk", bufs=3) as msk_pool,
            tc.tile_pool(name="mm", bufs=3) as mm_pool,
        ):
            d_pools = [d0_pool, d1_pool, d2_pool, d3_pool]
            dT_pools = [dT0_pool, dT1_pool, dT2_pool, dT3_pool]
            nc.gpsimd.load_library(library_config.mlp)
            nidx_reg = nc.gpsimd.to_reg(GCHUNK)

            idx_sb = io_pool.tile([P, N_GATHERS * IDX_COLS], mybir.dt.int16)
            nc.sync.dma_start(out=idx_sb[:], in_=idx[:])
            slab_sb = io_pool.tile([P, SLAB_COLS], bf16)
            nc.sync.dma_start(out=slab_sb[:], in_=slab[:])
            ident_sb = io_pool.tile([P, P], bf16)
            nc.sync.dma_start(out=ident_sb[:], in_=ident[:])
            scores = io_pool.tile([P, SCOL], f32)

            d_tiles = {}

            def emit_gathers(t):
                if t >= TILES_PER_BANK:
                    return
                for b in range(N_BANKS):
                    g = d_pools[b].tile([P, GCHUNK], bf16, tag=f"d{b}")
                    gi = N_BANKS * t + b
                    nc.gpsimd.dma_gather(
                        out_ap=g[:].rearrange("p (c e) -> p c e", e=D),
                        in_ap=h[b * BANK:(b + 1) * BANK, :],
                        idxs_ap=idx_sb[:, gi * IDX_COLS:(gi + 1) * IDX_COLS],
                        num_idxs=GCHUNK,
                        num_idxs_reg=nidx_reg,
                        elem_size=D,
                        transpose=False,
                        single_packet=True,
                        queue_num=b,
                    )
                    # transpose each 128-slot atom on PE, downcast to bf16
                    pt = psT_pool.tile([P, GCHUNK], bf16, tag="psT")
                    for a in range(APT):
                        nc.tensor.transpose(
                            pt[:, a * ATOM:(a + 1) * ATOM],
                            g[:, a * ATOM:(a + 1) * ATOM],
                            ident_sb[:],
                        )
                    gT = dT_pools[b].tile([P, GCHUNK], bf16, tag=f"dT{b}")
                    if b % 2 == 0:
                        nc.scalar.copy(out=gT[:], in_=pt[:])
                    else:
                        nc.vector.tensor_copy(gT[:], pt[:])
                    d_tiles[(b, t)] = gT

            for t in range(PREFETCH + 1):
                emit_gathers(t)

            ps = None
            for w in range(W):
                t = w // APT
                if w % APT == 0 and t >= 1:
                    emit_gathers(t + PREFETCH)
                if w % WB == 0:
                    ps = ps_pool.tile([P, BCOLS], f32)
                    mask = msk_pool.tile([P, BCOLS], bf16, tag="msk")
                    nc.sync.dma_start(
                        out=mask[:],
                        in_=mask_d[:, N_BANKS * ATOM * w:
                                   N_BANKS * ATOM * w + BCOLS],
                    )
                for b in range(N_BANKS):
                    col = ((w % WB) * N_BANKS + b) * ATOM
                    nc.tensor.matmul(
                        ps[:, col:col + ATOM],
                        d_tiles[(b, t)][:, (w % APT) * ATOM:
                                        (w % APT + 1) * ATOM],
                        slab_sb[:, S_WIN * w:S_WIN * w + ATOM],
                        start=True, stop=True,
                    )
                if w % WB == WB - 1:
                    w0 = w - (WB - 1)
                    nb = WB * N_BANKS
                    s_bf = sbf_pool.tile([P, BCOLS], bf16, tag="sbf")
                    nc.scalar.copy(out=s_bf[:], in_=ps[:])
                    mm = mm_pool.tile([P, BCOLS], bf16, tag="mm")
                    nc.vector.tensor_tensor(
                        out=mm[:], in0=s_bf[:], in1=mask[:],
                        op=mybir.AluOpType.mult,
                    )
                    nc.vector.tensor_reduce(
                        out=scores[:, N_BANKS * w0:N_BANKS * w0 + nb],
                        in_=mm[:].rearrange("p (a n) -> p a n", n=ATOM),
                        axis=mybir.AxisListType.X,
                        op=mybir.AluOpType.add,
                    )

            pmin = io_pool.tile([P, 1], f32)
            nc.vector.tensor_reduce(
                out=pmin[:], in_=scores[:], axis=mybir.AxisListType.X,
                op=mybir.AluOpType.min,
            )
            nc.sync.dma_start(out=pmin_d[:], in_=pmin[:])
            nc.gpsimd.collective_compute(
                "AllReduce",
                mybir.AluOpType.min,
                replica_groups=[list(range(N_CORES))],
                ins=[pmin_d[:]],
                outs=[gmin_d[:]],
            )
            gbc = io_pool.tile([P, P], f32)
            nc.sync.dma_start(
                out=gbc[:], in_=gmin_d[:, 0][None, :].to_broadcast((P, P))
            )
            gmin = io_pool.tile([P, 1], f32)
            nc.vector.tensor_reduce(
                out=gmin[:], in_=gbc[:], axis=mybir.AxisListType.X,
                op=mybir.AluOpType.min,
            )
            out_sb = io_pool.tile([P, SCOL], f32)
            nc.vector.tensor_scalar(
                out=out_sb[:], in0=scores[:], scalar1=gmin[:], scalar2=None,
                op0=mybir.AluOpType.not_equal,
            )
            nc.sync.dma_start(out=out[:], in_=out_sb[:])
            nc.sync.dma_start(out=sc_out[:], in_=scores[:])

    _split_multi_waits(nc)
    mybir.codegen_inst_isa_subclasses(nc)
    return nc


def _split_multi_waits(nc):
    """walrus rejects >1 sync-wait per ISA instruction; hoist extras onto
    standalone EventSemaphore instructions just before it (same engine)."""
    n = 0
    for blk in nc.m.functions[0].blocks:
        new_list = []
        for ins in blk.instructions:
            si = ins.sync_info
            if (
                si is not None
                and si.on_wait
                and len(si.on_wait) > 1
                and not isinstance(ins, mybir.InstEventSemaphore)
            ):
                waits = list(si.on_wait)
                for wt in waits[:-1]:
                    n += 1
                    ev = mybir.InstEventSemaphore(
                        name=f"wait_split_{n}",
                        opcode="EventSemaphore",
                        engine=ins.engine,
                        ins=[],
                        outs=[],
                        sync_info=mybir.SyncInfo(on_wait=[wt], on_update=[]),
                    )
                    nc.inst_map[ev.name] = ev
                    new_list.append(ev)
                si.on_wait = [waits[-1]]
            new_list.append(ins)
        blk.instructions[:] = new_list


def _plan_core(src_l, dst, epos):
    """Greedy-pack this core's edges into (window, bank) atoms.

    src_l: window-shard-local src (0..NPC-1), dst: global dst, epos: global
    edge positions. Returns (idx16 [128, N_GATHERS*IDX_COLS], srcl
    [128, SCOL] f32, slot_row/col per edge, overflow list, any_empty)."""
    db = dst // BANK
    dl = (dst % BANK).astype(np.int64)
    idx_bank = np.zeros((N_BANKS, SLOTS_PER_BANK), np.int16)
    srcl_arr = np.full((P, SCOL), PAD_SRCL, np.float32)
    slot_row = np.full(src_l.shape[0], -1, np.int64)
    slot_col = np.full(src_l.shape[0], -1, np.int64)
    overflow = []
    any_empty = False
    for b in range(N_BANKS):
        sel = np.nonzero(db == b)[0]
        order = sel[np.argsort(src_l[sel], kind="stable")]
        fill = np.zeros(W, np.int32)
        # per-window member lists
        w_of = np.full(order.shape[0], -1, np.int32)
        for k, e in enumerate(order):
            n = src_l[e]
            w_lo = max(0, -(-(int(n) - 127) // S_WIN))
            w_hi = min(W - 1, int(n) // S_WIN)
            for w in range(w_lo, w_hi + 1):
                if fill[w] < ATOM:
                    w_of[k] = w
                    slot_row[e] = fill[w]
                    slot_col[e] = N_BANKS * w + b
                    srcl_arr[fill[w], N_BANKS * w + b] = n - S_WIN * w
                    idx_bank[b, w * ATOM + fill[w]] = dl[e]
                    fill[w] += 1
                    break
            else:
                overflow.append(int(epos[e]))
        # pad each atom by duplicating its first member (score dup = safe)
        for w in range(W):
            k = fill[w]
            if k == 0:
                any_empty = True  # pad stays (dl=0, srcl=PAD) -> score 0
                continue
            if k < ATOM:
                idx_bank[b, w * ATOM + k:(w + 1) * ATOM] = idx_bank[b, w * ATOM]
                srcl_arr[k:ATOM, N_BANKS * w + b] = srcl_arr[0, N_BANKS * w + b]
    # wrap idx into the ucode's [16, IDX_COLS] layout per gather tile
    idx16 = np.empty((16, N_GATHERS * IDX_COLS), np.int16)
    for t in range(TILES_PER_BANK):
        for b in range(N_BANKS):
            gi = N_BANKS * t + b
            vals = idx_bank[b, t * GCHUNK:(t + 1) * GCHUNK]
            idx16[:, gi * IDX_COLS:(gi + 1) * IDX_COLS] = (
                vals.reshape(IDX_COLS, 16).T
            )
    idx16_full = np.tile(idx16, (8, 1))
    return idx16_full, srcl_arr, slot_row, slot_col, overflow, any_empty


def make_in_maps(h, src, dst):
    import ml_dtypes
    h16 = np.ascontiguousarray(
        np.asarray(h, dtype=np.float32).astype(ml_dtypes.bfloat16)
    )
    src64 = np.asarray(src, dtype=np.int64)
    dst64 = np.asarray(dst, dtype=np.int64)
    ident_arr = np.ascontiguousarray(np.eye(P, dtype=np.float32)
                                     .astype(ml_dtypes.bfloat16))
    in_maps, plans = [], []
    for c in range(N_CORES):
        epos = np.nonzero((src64 >= c * NPC) & (src64 < (c + 1) * NPC))[0]
        src_l = src64[epos] - c * NPC
        idx16, srcl_arr, slot_row, slot_col, overflow, any_empty = _plan_core(
            src_l, dst64[epos], epos
        )
        slab = np.zeros((P, SLAB_COLS), np.float32)
        slab[:, :NPC] = np.asarray(
            h16[c * NPC:(c + 1) * NPC, :], dtype=np.float32
        ).T
        in_maps.append({
            "h": h16,
            "slab": np.ascontiguousarray(slab.astype(ml_dtypes.bfloat16)),
            "idx": np.ascontiguousarray(idx16),
            "mask": np.ascontiguousarray(
                (srcl_arr[:, :, None] == np.arange(ATOM, dtype=np.float32))
                .astype(ml_dtypes.bfloat16).reshape(P, SCOL * ATOM)),
            "ident": ident_arr,
        })
        plans.append((epos, slot_row, slot_col, overflow, any_empty))
    return in_maps, plans


def assemble_output(results, plans, h):
    out = np.empty((E_TOTAL,), np.float32)
    all_of = [(c, pos) for c, p in enumerate(plans) for pos in p[3]]
    force_host = bool(all_of) or any(p[4] for p in plans)
    if not force_host:
        for (epos, srow, scol, _, _), r in zip(plans, results):
            out[epos] = r["out"][srow, scol]
        return out.reshape(E_TOTAL, 1)
    # host fallback: recompute global min incl. overflow edges, re-threshold
    import ml_dtypes
    h16 = np.asarray(h, dtype=np.float32).astype(ml_dtypes.bfloat16)
    h32 = np.asarray(h16, dtype=np.float32)
    src, dst = _ASSEMBLE_SRC
    scores = np.empty((E_TOTAL,), np.float32)
    for (epos, srow, scol, overflow, _), r in zip(plans, results):
        scores[epos] = r["sc"][srow, scol]
        for pos in overflow:
            scores[pos] = float(h32[src[pos]] @ h32[dst[pos]])
    gmin = float(scores.min())
    out = (scores != gmin).astype(np.float32)
    return out.reshape(E_TOTAL, 1)


_ASSEMBLE_SRC = [None, None]


def kernel(h, src, dst):
    if "nc" not in _CACHE:
        _CACHE["nc"] = build_nc()
    nc = _CACHE["nc"]
    _ASSEMBLE_SRC[0] = np.asarray(src, dtype=np.int64)
    _ASSEMBLE_SRC[1] = np.asarray(dst, dtype=np.int64)
    in_maps, plans = make_in_maps(h, src, dst)
    res = run_bass_kernel_spmd(nc, in_maps, list(range(N_CORES)))
    return assemble_output(res.results, plans, h)
